# revision 7
# baseline (speedup 1.0000x reference)
"""Int8-quantized linear: y = x @ (w_q * scale)^T + bias, tensor-parallel on 8 cores.

Shapes (hardcoded): x [4,32,4096] f32, w_q [11008,4096] int8, scale [1] f32,
bias [11008] f32 -> out [4,32,11008] f32.

Strategy: column-parallel over out_features (1376 per core). The int8 weight
shard is pre-arranged on host to [128, 32*1376] (partition p, col block t =
w^T[t*128+p, :]) so every DMA transfer is per-partition contiguous in DRAM
(1.4-5.5KB descriptors). The measured pacer at this size is the weight DMA
itself (~284 GB/s on one HWDGE ring -> ~20us for 5.6MB), so v3 alternates
weight transfers between BOTH HWDGE rings (SP + ACT) and tapers the group
sizes [1,1,2,4,4,...,4,2,1,1]: big groups mid-stream for trigger efficiency,
small groups at both ends so neither the first matmul nor the post-last-byte
tail (conversion -> matmuls -> evict -> out-DMA) sits behind a 1.4MB batch.

The PE runs fp16 matmuls with x (scale folded, fp16) as the stationary
operand. int8->fp16 upconvert is split DVE 920 cols/chunk (2x_2P copy) /
ACT 456 (ACTIVATE copy), one batched 3D-AP instruction per DMA group to
amortize fixed costs (DVE 150cyc, ACT 352cyc). Three PSUM banks (512+408
DVE / 456 ACT) keep every matmul at exactly 2 sync-waits (x sem, converter
sem) - this walrus allows 2 per compute op, 1 per DMA. Bias enters PSUM via
a K=2 ones-matmul (fp16 hi+lo). PSUM is evicted by DVE (A,B) and ACT (C)
into an fp16 SBUF tile (|y| <= ~40, fp16 out adds ~4e-4 rel err vs the 2e-2
gate) and DMA'd out on gpsimd SWDGE in two transfers split at the writer
boundary; x + bias also ride SWDGE so the HWDGE rings carry only weights.
Host concatenates the 8 shards and upcasts to fp32.
"""

import numpy as np

P = 128            # partitions = B*S tokens
IN_F = 4096
OUT_F = 11008
N_CORES = 8
N_SHARD = OUT_F // N_CORES          # 1376
K_CHUNKS = IN_F // P                # 32
COLS_D = 912       # DVE-converted cols per chunk
COLS_A = N_SHARD - COLS_D           # 464, ACT-converted
BANKS_D = [(0, 512), (512, COLS_D - 512)]   # (offset within DVE tile, size)
# k-chunks per weight DMA transfer on the SP HWDGE ring: taper at both
# ends (first matmul starts early; post-last-byte tail is one chunk deep)
WGROUPS = [1, 1, 2, 4, 8, 8, 4, 2, 1, 1]
# x DMA transfers (k-chunks each) on the ACT HWDGE ring: SWDGE mid-stream
# poisons HWDGE throughput (descriptor rings live on SBUF AXI ports), so
# only the tail out-DMAs use SWDGE
XKS = [16, 16]

_CACHE = {}


def _patch_tile_drain():
    """The walrus build in this env rejects >2 sync-wait commands on one
    instruction; Tile's kernel-tail drain aggregates one wait per live
    semaphore. Re-emit the tail as one single-wait drain per outstanding
    proc (semantically identical: SP serially waits each sem, then the
    usual all-engine barrier runs)."""
    import concourse.tile as tile
    from concourse.vector_clock import ScopedClock, VectorClock

    if getattr(tile.TileContext, "_ant_drain_patched", False):
        return
    N_PROCS = 27

    def _drain_and_barrier(self, tick_clock, wait_clock):
        gc = tick_clock.global_clock
        live = [p for p in range(N_PROCS) if gc[p] > 0]
        for p in live:
            vc = VectorClock([gc[q] if q == p else 0 for q in range(N_PROCS)])
            d = self.nc.sync.drain()
            wait_clock.add_sem_waits(d.ins, ScopedClock({None: vc}))
        if not live:
            self.nc.sync.drain()
        self.nc.all_engine_barrier()
        assert self.sems is not None
        popped = self.nc._tile_sem_poison_stack.pop()
        assert popped is self._sem_poison
        self.nc.clear_and_free_semaphores(list(self.sems.allocated().values()))
        self.nc.all_engine_barrier()

    tile.TileContext._drain_and_barrier = _drain_and_barrier
    tile.TileContext._ant_drain_patched = True


def _build_nc():
    import concourse.bass as bass
    import concourse.mybir as mybir
    import concourse.tile as tile

    _patch_tile_drain()
    nc = bass.Bass()
    xs = nc.declare_dram_parameter("xs", [P, IN_F], mybir.dt.float16, isOutput=False)
    wq = nc.declare_dram_parameter(
        "wq", [P, K_CHUNKS * N_SHARD], mybir.dt.int8, isOutput=False)
    bi = nc.declare_dram_parameter("bias2", [2, N_SHARD], mybir.dt.float16, isOutput=False)
    out = nc.declare_dram_parameter("out", [P, N_SHARD], mybir.dt.float16, isOutput=True)

    with tile.TileContext(nc) as tc:
        # bufs=1 everywhere: every tile below has a distinct tag (they all
        # coexist; no slot recycling, so no WAR/WAW sync waits)
        with tc.tile_pool(name="const", bufs=1) as cpool, \
             tc.tile_pool(name="w8", bufs=1) as w8p, \
             tc.tile_pool(name="w16d", bufs=1) as w16dp, \
             tc.tile_pool(name="w16a", bufs=1) as w16ap, \
             tc.tile_pool(name="ps", bufs=1, space="PSUM") as psp, \
             tc.tile_pool(name="ob", bufs=1) as obp:
            NX = len(XKS)
            xko = [sum(XKS[:i]) for i in range(NX + 1)]
            xts = [cpool.tile([P, XKS[i] * P], mybir.dt.float16,
                              name=f"xq{i}", tag=f"xq{i}") for i in range(NX)]

            def xslice(k):
                i = next(i for i in range(NX) if xko[i] <= k < xko[i + 1])
                o = (k - xko[i]) * P
                return xts[i][:, o:o + P]

            ones_t = cpool.tile([2, P], mybir.dt.float16)
            nc.vector.memset(ones_t[:], 1.0)
            bias_t = cpool.tile([2, N_SHARD], mybir.dt.float16)
            warm_t = cpool.tile([P, 512], mybir.dt.float16)
            nc.vector.memset(warm_t[:], 0.0)

            # x + bias on the ACT HWDGE ring (runs concurrent with the SP
            # weight stream inside the ~435 GB/s SDMA pool)
            for i in range(NX):
                nc.scalar.dma_start(
                    out=xts[i][:], in_=xs[:, xko[i] * P:xko[i + 1] * P])
            nc.scalar.dma_start(out=bias_t[:], in_=bi[:])

            # weight stream on the SP HWDGE ring: 2D contiguous slices of
            # the host-prearranged [P, 32*1376] layout
            wtiles = []
            k = 0
            for g, gsz in enumerate(WGROUPS):
                w8 = w8p.tile([P, gsz * N_SHARD], mybir.dt.int8,
                              name=f"w8_{g}", tag=f"w8_{g}")
                nc.sync.dma_start(
                    out=w8[:], in_=wq[:, k * N_SHARD:(k + gsz) * N_SHARD])
                wtiles.append((w8, k, gsz))
                k += gsz

            psW = psp.tile([P, 512], mybir.dt.float32, name="psW", tag="psW")
            # prewarm: keep the PE busy through the DMA lead-in so the HAM
            # clock gate reaches 8/8 (2.4 GHz) before the real stream starts
            for _ in range(7):
                nc.tensor.matmul(psW[:], lhsT=warm_t[:, 0:P], rhs=warm_t[:],
                                 start=True, stop=True)
            psA = psp.tile([P, 512], mybir.dt.float32, name="psA", tag="psA")
            psB = psp.tile([P, COLS_D - 512], mybir.dt.float32, name="psB", tag="psB")
            psC = psp.tile([P, COLS_A], mybir.dt.float32, name="psC", tag="psC")

            for w8, k0, gsz in wtiles:
                # one batched conversion per engine per DMA group (3D APs:
                # [p][t][cols]); amortizes DVE's 150cyc / ACT's 352cyc fixed
                # cost and keeps sem traffic low
                src3 = w8[:].rearrange("p (t n) -> p t n", t=gsz)
                w16d = w16dp.tile([P, gsz * COLS_D], mybir.dt.float16,
                                  name=f"w16d_{k0}", tag=f"w16d_{k0}")
                nc.vector.tensor_copy(
                    out=w16d[:].rearrange("p (t n) -> p t n", t=gsz),
                    in_=src3[:, :, 0:COLS_D])
                w16a = w16ap.tile([P, gsz * COLS_A], mybir.dt.float16,
                                  name=f"w16a_{k0}", tag=f"w16a_{k0}")
                nc.scalar.copy(
                    out=w16a[:].rearrange("p (t n) -> p t n", t=gsz),
                    in_=src3[:, :, COLS_D:N_SHARD])
                for t in range(gsz):
                    kk = k0 + t
                    xsl = xslice(kk)
                    st, sp = (kk == 0), (kk == K_CHUNKS - 1)
                    for off, sz in BANKS_D:
                        dst = psA if off == 0 else psB
                        nc.tensor.matmul(
                            dst[:], lhsT=xsl,
                            rhs=w16d[:, t * COLS_D + off:t * COLS_D + off + sz],
                            start=st, stop=sp)
                    nc.tensor.matmul(
                        psC[:], lhsT=xsl,
                        rhs=w16a[:, t * COLS_A:(t + 1) * COLS_A],
                        start=st, stop=sp)
                    if kk == 16:
                        # bias mid-stream (accumulation order is irrelevant):
                        # psum[m, n] += 1*b_hi[n] + 1*b_lo[n]
                        for off, sz in BANKS_D:
                            dst = psA if off == 0 else psB
                            nc.tensor.matmul(
                                dst[:], lhsT=ones_t[:],
                                rhs=bias_t[:, off:off + sz],
                                start=False, stop=False)
                        nc.tensor.matmul(
                            psC[:], lhsT=ones_t[:],
                            rhs=bias_t[:, COLS_D:N_SHARD],
                            start=False, stop=False)
            # eviction split by engine (fp32 psum -> fp16 staging); out-DMA
            # boundaries align with the writer split so each SWDGE DMA waits
            # on exactly one engine (walrus allows 1 wait per DMA)
            ob = obp.tile([P, N_SHARD], mybir.dt.float16)
            nc.vector.tensor_copy(out=ob[:, 0:512], in_=psA[:])
            nc.vector.tensor_copy(out=ob[:, 512:COLS_D], in_=psB[:])
            nc.scalar.copy(out=ob[:, COLS_D:], in_=psC[:])
            nc.gpsimd.dma_start(out=out[:, :COLS_D], in_=ob[:, :COLS_D])
            nc.gpsimd.dma_start(out=out[:, COLS_D:], in_=ob[:, COLS_D:])
    return nc


def get_nc():
    if "nc" not in _CACHE:
        _CACHE["nc"] = _build_nc()
    return _CACHE["nc"]


def make_in_maps(x, w_q, scale, bias):
    """Host-side shard/layout prep. Returns list of 8 per-core input dicts."""
    x = np.asarray(x, dtype=np.float32).reshape(P, IN_F)
    s = float(np.asarray(scale).reshape(-1)[0])
    xs = (x * s).astype(np.float16)
    # SBUF layout: x_sb[p, nk*128+m] = xs[m, nk*128+p] (contraction on partitions)
    x_sb = np.ascontiguousarray(
        xs.reshape(P, K_CHUNKS, P).transpose(2, 1, 0)
    ).reshape(P, IN_F)

    w8 = np.asarray(w_q).astype(np.int8)
    wT = w8.T  # [IN_F, OUT_F]

    b32 = np.asarray(bias, dtype=np.float32)
    b_hi = b32.astype(np.float16)
    b_lo = (b32 - b_hi.astype(np.float32)).astype(np.float16)

    in_maps = []
    for c in range(N_CORES):
        lo, hi = c * N_SHARD, (c + 1) * N_SHARD
        # [P, 32*1376]: partition p, col block t = wT[t*128+p, lo:hi] so any
        # group of k-chunks is per-partition contiguous in DRAM
        shard = np.ascontiguousarray(
            wT[:, lo:hi].reshape(K_CHUNKS, P, N_SHARD).transpose(1, 0, 2)
        ).reshape(P, K_CHUNKS * N_SHARD)
        in_maps.append({
            "xs": x_sb,
            "wq": shard,
            "bias2": np.ascontiguousarray(
                np.stack([b_hi[lo:hi], b_lo[lo:hi]], axis=0)
            ),
        })
    return in_maps


def gather(results):
    """results: list of 8 dicts with 'out' [P, N_SHARD] f16 -> full f32 output."""
    full = np.concatenate(
        [np.asarray(r["out"]).astype(np.float32) for r in results], axis=1)
    return np.ascontiguousarray(full.reshape(4, 32, OUT_F))


def kernel(x, w_q, scale, bias):
    from concourse.bass_utils import run_bass_kernel_spmd

    nc = get_nc()
    in_maps = make_in_maps(x, w_q, scale, bias)
    res = run_bass_kernel_spmd(nc, in_maps, list(range(N_CORES)))
    return gather(res.results)


# revision 8
# speedup vs baseline: 1.0578x; 1.0578x over previous
"""Int8-quantized linear: y = x @ (w_q * scale)^T + bias, tensor-parallel on 8 cores.

Shapes (hardcoded): x [4,32,4096] f32, w_q [11008,4096] int8, scale [1] f32,
bias [11008] f32 -> out [4,32,11008] f32.

Strategy: column-parallel over out_features (1376 per core). The int8 weight
shard is pre-arranged on host to [128, 32*1376] (partition p, col block t =
w^T[t*128+p, :]) so every DMA transfer is per-partition contiguous in DRAM
(1.4-5.5KB descriptors). The measured pacer at this size is the weight DMA
itself (~284 GB/s on one HWDGE ring -> ~20us for 5.6MB), so v3 alternates
weight transfers between BOTH HWDGE rings (SP + ACT) and tapers the group
sizes [1,1,2,4,4,...,4,2,1,1]: big groups mid-stream for trigger efficiency,
small groups at both ends so neither the first matmul nor the post-last-byte
tail (conversion -> matmuls -> evict -> out-DMA) sits behind a 1.4MB batch.

The PE runs fp16 matmuls with x (scale folded, fp16) as the stationary
operand. int8->fp16 upconvert is split DVE 920 cols/chunk (2x_2P copy) /
ACT 456 (ACTIVATE copy), one batched 3D-AP instruction per DMA group to
amortize fixed costs (DVE 150cyc, ACT 352cyc). Three PSUM banks (512+408
DVE / 456 ACT) keep every matmul at exactly 2 sync-waits (x sem, converter
sem) - this walrus allows 2 per compute op, 1 per DMA. Bias enters PSUM via
a K=2 ones-matmul (fp16 hi+lo). PSUM is evicted by DVE (A,B) and ACT (C)
into an fp16 SBUF tile (|y| <= ~40, fp16 out adds ~4e-4 rel err vs the 2e-2
gate) and DMA'd out on gpsimd SWDGE in two transfers split at the writer
boundary; x + bias also ride SWDGE so the HWDGE rings carry only weights.
Host concatenates the 8 shards and upcasts to fp32.
"""

import numpy as np

P = 128            # partitions = B*S tokens
IN_F = 4096
OUT_F = 11008
N_CORES = 8
N_SHARD = OUT_F // N_CORES          # 1376
K_CHUNKS = IN_F // P                # 32
COLS_D = 912       # DVE-converted cols per chunk
COLS_A = N_SHARD - COLS_D           # 464, ACT-converted
BANKS_D = [(0, 512), (512, COLS_D - 512)]   # (offset within DVE tile, size)
# k-chunks per weight DMA transfer on the SP HWDGE ring: taper at both
# ends (first matmul starts early; post-last-byte tail is one chunk deep)
WGROUPS = [1, 2, 4, 8, 8, 4, 2, 1, 1, 1]
# x DMA transfers (k-chunks each) on the ACT HWDGE ring: SWDGE mid-stream
# poisons HWDGE throughput (descriptor rings live on SBUF AXI ports), so
# only the tail out-DMAs use SWDGE
XKS = [16, 16]

_CACHE = {}


def _patch_tile_drain():
    """The walrus build in this env rejects >2 sync-wait commands on one
    instruction; Tile's kernel-tail drain aggregates one wait per live
    semaphore. Re-emit the tail as one single-wait drain per outstanding
    proc (semantically identical: SP serially waits each sem, then the
    usual all-engine barrier runs)."""
    import concourse.tile as tile
    from concourse.vector_clock import ScopedClock, VectorClock

    if getattr(tile.TileContext, "_ant_drain_patched", False):
        return
    N_PROCS = 27

    def _drain_and_barrier(self, tick_clock, wait_clock):
        gc = tick_clock.global_clock
        live = [p for p in range(N_PROCS) if gc[p] > 0]
        for p in live:
            vc = VectorClock([gc[q] if q == p else 0 for q in range(N_PROCS)])
            d = self.nc.sync.drain()
            wait_clock.add_sem_waits(d.ins, ScopedClock({None: vc}))
        if not live:
            self.nc.sync.drain()
        self.nc.all_engine_barrier()
        assert self.sems is not None
        popped = self.nc._tile_sem_poison_stack.pop()
        assert popped is self._sem_poison
        self.nc.clear_and_free_semaphores(list(self.sems.allocated().values()))
        self.nc.all_engine_barrier()

    tile.TileContext._drain_and_barrier = _drain_and_barrier
    tile.TileContext._ant_drain_patched = True


def _build_nc():
    import concourse.bass as bass
    import concourse.mybir as mybir
    import concourse.tile as tile

    _patch_tile_drain()
    nc = bass.Bass()
    xs = nc.declare_dram_parameter("xs", [P, IN_F], mybir.dt.float16, isOutput=False)
    wq = nc.declare_dram_parameter(
        "wq", [P, K_CHUNKS * N_SHARD], mybir.dt.int8, isOutput=False)
    bi = nc.declare_dram_parameter("bias2", [2, N_SHARD], mybir.dt.float16, isOutput=False)
    out = nc.declare_dram_parameter("out", [P, N_SHARD], mybir.dt.float16, isOutput=True)

    with tile.TileContext(nc) as tc:
        # bufs=1 everywhere: every tile below has a distinct tag (they all
        # coexist; no slot recycling, so no WAR/WAW sync waits)
        with tc.tile_pool(name="const", bufs=1) as cpool, \
             tc.tile_pool(name="w8", bufs=1) as w8p, \
             tc.tile_pool(name="w16d", bufs=1) as w16dp, \
             tc.tile_pool(name="w16a", bufs=1) as w16ap, \
             tc.tile_pool(name="ps", bufs=1, space="PSUM") as psp, \
             tc.tile_pool(name="ob", bufs=1) as obp:
            NX = len(XKS)
            xko = [sum(XKS[:i]) for i in range(NX + 1)]
            xts = [cpool.tile([P, XKS[i] * P], mybir.dt.float16,
                              name=f"xq{i}", tag=f"xq{i}") for i in range(NX)]

            def xslice(k):
                i = next(i for i in range(NX) if xko[i] <= k < xko[i + 1])
                o = (k - xko[i]) * P
                return xts[i][:, o:o + P]

            ones_t = cpool.tile([2, P], mybir.dt.float16)
            nc.vector.memset(ones_t[:], 1.0)
            bias_t = cpool.tile([2, N_SHARD], mybir.dt.float16)
            warm_t = cpool.tile([P, 512], mybir.dt.float16)
            nc.vector.memset(warm_t[:], 0.0)

            # x + bias on the ACT HWDGE ring (runs concurrent with the SP
            # weight stream inside the ~435 GB/s SDMA pool)
            for i in range(NX):
                nc.scalar.dma_start(
                    out=xts[i][:], in_=xs[:, xko[i] * P:xko[i + 1] * P])
            nc.scalar.dma_start(out=bias_t[:], in_=bi[:])

            # weight stream on the SP HWDGE ring: 2D contiguous slices of
            # the host-prearranged [P, 32*1376] layout
            wtiles = []
            k = 0
            for g, gsz in enumerate(WGROUPS):
                w8 = w8p.tile([P, gsz * N_SHARD], mybir.dt.int8,
                              name=f"w8_{g}", tag=f"w8_{g}")
                nc.sync.dma_start(
                    out=w8[:], in_=wq[:, k * N_SHARD:(k + gsz) * N_SHARD])
                wtiles.append((w8, k, gsz))
                k += gsz

            psW = psp.tile([P, 512], mybir.dt.float32, name="psW", tag="psW")
            # prewarm: keep the PE busy through the DMA lead-in so the HAM
            # clock gate reaches 8/8 (2.4 GHz) before the real stream starts
            for _ in range(4):
                nc.tensor.matmul(psW[:], lhsT=warm_t[:, 0:P], rhs=warm_t[:],
                                 start=True, stop=True)
            psA = psp.tile([P, 512], mybir.dt.float32, name="psA", tag="psA")
            psB = psp.tile([P, COLS_D - 512], mybir.dt.float32, name="psB", tag="psB")
            psC = psp.tile([P, COLS_A], mybir.dt.float32, name="psC", tag="psC")

            for w8, k0, gsz in wtiles:
                # one batched conversion per engine per DMA group (3D APs:
                # [p][t][cols]); amortizes DVE's 150cyc / ACT's 352cyc fixed
                # cost and keeps sem traffic low
                src3 = w8[:].rearrange("p (t n) -> p t n", t=gsz)
                w16d = w16dp.tile([P, gsz * COLS_D], mybir.dt.float16,
                                  name=f"w16d_{k0}", tag=f"w16d_{k0}")
                nc.vector.tensor_copy(
                    out=w16d[:].rearrange("p (t n) -> p t n", t=gsz),
                    in_=src3[:, :, 0:COLS_D])
                w16a = w16ap.tile([P, gsz * COLS_A], mybir.dt.float16,
                                  name=f"w16a_{k0}", tag=f"w16a_{k0}")
                nc.scalar.copy(
                    out=w16a[:].rearrange("p (t n) -> p t n", t=gsz),
                    in_=src3[:, :, COLS_D:N_SHARD])
                for t in range(gsz):
                    kk = k0 + t
                    xsl = xslice(kk)
                    st, sp = (kk == 0), (kk == K_CHUNKS - 1)
                    for off, sz in BANKS_D:
                        dst = psA if off == 0 else psB
                        nc.tensor.matmul(
                            dst[:], lhsT=xsl,
                            rhs=w16d[:, t * COLS_D + off:t * COLS_D + off + sz],
                            start=st, stop=sp)
                    nc.tensor.matmul(
                        psC[:], lhsT=xsl,
                        rhs=w16a[:, t * COLS_A:(t + 1) * COLS_A],
                        start=st, stop=sp)
                    if kk == 16:
                        # bias mid-stream (accumulation order is irrelevant):
                        # psum[m, n] += 1*b_hi[n] + 1*b_lo[n]
                        for off, sz in BANKS_D:
                            dst = psA if off == 0 else psB
                            nc.tensor.matmul(
                                dst[:], lhsT=ones_t[:],
                                rhs=bias_t[:, off:off + sz],
                                start=False, stop=False)
                        nc.tensor.matmul(
                            psC[:], lhsT=ones_t[:],
                            rhs=bias_t[:, COLS_D:N_SHARD],
                            start=False, stop=False)
            # eviction split by engine (fp32 psum -> fp16 staging); out-DMA
            # boundaries align with the writer split so each SWDGE DMA waits
            # on exactly one engine (walrus allows 1 wait per DMA)
            ob = obp.tile([P, N_SHARD], mybir.dt.float16)
            nc.vector.tensor_copy(out=ob[:, 0:512], in_=psA[:])
            nc.vector.tensor_copy(out=ob[:, 512:COLS_D], in_=psB[:])
            nc.scalar.copy(out=ob[:, COLS_D:], in_=psC[:])
            nc.gpsimd.dma_start(out=out[:, :COLS_D], in_=ob[:, :COLS_D])
            nc.gpsimd.dma_start(out=out[:, COLS_D:], in_=ob[:, COLS_D:])
    return nc


def get_nc():
    if "nc" not in _CACHE:
        _CACHE["nc"] = _build_nc()
    return _CACHE["nc"]


def make_in_maps(x, w_q, scale, bias):
    """Host-side shard/layout prep. Returns list of 8 per-core input dicts."""
    x = np.asarray(x, dtype=np.float32).reshape(P, IN_F)
    s = float(np.asarray(scale).reshape(-1)[0])
    xs = (x * s).astype(np.float16)
    # SBUF layout: x_sb[p, nk*128+m] = xs[m, nk*128+p] (contraction on partitions)
    x_sb = np.ascontiguousarray(
        xs.reshape(P, K_CHUNKS, P).transpose(2, 1, 0)
    ).reshape(P, IN_F)

    w8 = np.asarray(w_q).astype(np.int8)
    wT = w8.T  # [IN_F, OUT_F]

    b32 = np.asarray(bias, dtype=np.float32)
    b_hi = b32.astype(np.float16)
    b_lo = (b32 - b_hi.astype(np.float32)).astype(np.float16)

    in_maps = []
    for c in range(N_CORES):
        lo, hi = c * N_SHARD, (c + 1) * N_SHARD
        # [P, 32*1376]: partition p, col block t = wT[t*128+p, lo:hi] so any
        # group of k-chunks is per-partition contiguous in DRAM
        shard = np.ascontiguousarray(
            wT[:, lo:hi].reshape(K_CHUNKS, P, N_SHARD).transpose(1, 0, 2)
        ).reshape(P, K_CHUNKS * N_SHARD)
        in_maps.append({
            "xs": x_sb,
            "wq": shard,
            "bias2": np.ascontiguousarray(
                np.stack([b_hi[lo:hi], b_lo[lo:hi]], axis=0)
            ),
        })
    return in_maps


def gather(results):
    """results: list of 8 dicts with 'out' [P, N_SHARD] f16 -> full f32 output."""
    full = np.concatenate(
        [np.asarray(r["out"]).astype(np.float32) for r in results], axis=1)
    return np.ascontiguousarray(full.reshape(4, 32, OUT_F))


def kernel(x, w_q, scale, bias):
    from concourse.bass_utils import run_bass_kernel_spmd

    nc = get_nc()
    in_maps = make_in_maps(x, w_q, scale, bias)
    res = run_bass_kernel_spmd(nc, in_maps, list(range(N_CORES)))
    return gather(res.results)
